# revision 2
# baseline (speedup 1.0000x reference)
"""3-layer GCN on 8 TRN2 NeuronCores — v2.

Changes vs v1 baseline (1.86ms):
  * bf16 features stored in 256B-padded rows ([Np, 128] bf16, cols 0:64
    meaningful) so each edge gather is ONE descriptor for ONE node row —
    drops the pair-row/parity-list machinery.
  * sources split into two halves (A = windows 0..23, B = 24..48) with
    separate AllGather buffers: keeps gather indices in int16 range AND
    lets AG-A fire mid-aggregation / AG-B at layer end, fully hidden
    behind gather descriptor generation (the GPSIMD wall).
  * S one-hot matrices precomputed on HOST and streamed from DRAM
    (eliminates 426us of DVE IS_EQ work).
  * eager, explicitly scheduled gather issue (A-prefix then interleave)
    with deep buffering to keep GPSIMD back-to-back.
  * per-window incremental stores of the next layer's features.
  * trailing pads of each gather batch marked idx=-1 (HW trims them).

Self-contained: only numpy/ml_dtypes + the concourse stack.
"""
import math
import os
import sys
import types

import numpy as np
import ml_dtypes

BF16 = ml_dtypes.bfloat16

for _p in ("/opt/trn_rl_repo",):
    if _p not in sys.path and os.path.isdir(_p):
        sys.path.insert(0, _p)

import concourse.bacc as bacc
import concourse.bass as bass
import concourse.mybir as mybir
from concourse import tile
from concourse.bass_utils import run_bass_kernel_spmd

F32 = mybir.dt.float32
BF = mybir.dt.bfloat16
I16 = mybir.dt.int16

NCORES = 8
W = 128
CH = 64
OUTC = 16
NB = int(os.environ.get("K_NB", "24"))      # chunks per gather batch
AHEAD_A = int(os.environ.get("K_AHEAD_A", "4"))   # windows of A-prefetch
AHEAD_B = int(os.environ.get("K_AHEAD_B", "2"))
BUFS_A = int(os.environ.get("K_BUFS_A", "6"))
BUFS_B = int(os.environ.get("K_BUFS_B", "3"))
PREFIX_A = int(os.environ.get("K_PREFIX_A", "6"))  # A-batches forced at layer start
SGEN = os.environ.get("K_SGEN", "ve")              # 've' = IS_EQ on DVE, 'dma' = host S

LAST_PERF = None


def _install_ntff_hook():
    if "antenv.axon_hooks" in sys.modules:
        return
    try:
        from trn_agent_boot.trn_boot import _ntff_profile_via_ctypes

        mod = types.ModuleType("antenv.axon_hooks")
        box = [None]
        mod.set_axon_ntff_profile_hook = lambda h: box.__setitem__(0, h)
        mod.get_axon_ntff_profile_hook = lambda: box[0]
        mod.set_axon_ntff_profile_hook(
            _ntff_profile_via_ctypes("/opt/axon/libaxon_pjrt.so")
        )
        sys.modules["antenv.axon_hooks"] = mod
    except Exception:
        pass


def _prep(x, edge_index):
    N = x.shape[0]
    E = edge_index.shape[1]
    INC = x.shape[1]
    NPC = N // NCORES                  # 6250
    NW = -(-NPC // W)                  # 49
    NPCp = NW * W                      # 6272
    Np = NPCp * NCORES                 # 50176
    NWA = NW // 2                      # 24 windows -> class A
    NWB = NW - NWA                     # 25 windows -> class B
    NPCa = NWA * W                     # 3072
    NPCb = NWB * W                     # 3200

    src = np.ascontiguousarray(edge_index[0]).astype(np.int64)
    dst = np.ascontiguousarray(edge_index[1]).astype(np.int64)

    deg = 1.0 + np.bincount(dst, minlength=N).astype(np.float64)
    dinv = (1.0 / np.sqrt(deg)).astype(np.float32)

    # degree-sorted round-robin deal (keeps per-window counts balanced
    # across cores): rank r -> core (r//W)%8, window r//(W*8)
    order = np.argsort(-deg, kind="stable")
    r = np.arange(N)
    new_of_rank = ((r // W) % NCORES) * NPCp + (r // (W * NCORES)) * W + (r % W)
    newid = np.empty(N, np.int64)
    newid[order] = new_of_rank

    s_new = newid[src]
    d_new = newid[dst]
    score, sloc = np.divmod(s_new, NPCp)
    cls = (sloc >= NPCa).astype(np.int64)          # 0=A, 1=B
    gidx = np.where(cls == 0, score * NPCa + sloc,
                    score * NPCb + (sloc - NPCa))
    core, dloc = np.divmod(d_new, NPCp)
    win = dloc // W
    rel = (dloc % W).astype(np.int16)

    # chunk schedule shared across cores (SPMD): per (win, cls)
    cnt = np.zeros((NCORES, NW, 2), np.int64)
    np.add.at(cnt, (core, win, cls), 1)
    chmax = -(-cnt.max(axis=0) // 128)             # [NW, 2]
    base = np.zeros((NW, 2), np.int64)
    Cl = [0, 0]
    for l in (0, 1):
        base[:, l] = np.cumsum(chmax[:, l]) - chmax[:, l]
        Cl[l] = int(chmax[:, l].sum())

    # slot assignment: vectorized cumcount within (core, cls, win)
    key = (core * 2 + cls) * NW + win
    o = np.argsort(key, kind="stable")
    ks = key[o]
    new_grp = np.empty(E, np.bool_)
    new_grp[0] = True
    new_grp[1:] = ks[1:] != ks[:-1]
    starts = np.nonzero(new_grp)[0]
    grp_of = np.cumsum(new_grp) - 1
    cumcount = np.arange(E) - starts[grp_of]
    slot_sorted = base[win[o], cls[o]] * 128 + cumcount

    PADG = np.int64(32767)
    gi = [np.full((NCORES, Cl[l] * 128), PADG, np.int64) for l in (0, 1)]
    dr = [np.full((NCORES, Cl[l] * 128), 128, np.int16) for l in (0, 1)]
    for l in (0, 1):
        m = cls[o] == l
        gi[l][core[o][m], slot_sorted[m]] = gidx[o][m]
        dr[l][core[o][m], slot_sorted[m]] = rel[o][m]

    # sort slots within each 128-chunk by gather index (HBM locality);
    # pads (gidx=32767) go last within the chunk
    for l in (0, 1):
        g3 = gi[l].reshape(NCORES, Cl[l], 128)
        d3 = dr[l].reshape(NCORES, Cl[l], 128)
        srt = np.argsort(g3, axis=2, kind="stable")
        gi[l] = np.take_along_axis(g3, srt, axis=2).reshape(NCORES, -1)
        dr[l] = np.take_along_axis(d3, srt, axis=2).reshape(NCORES, -1)

    # batch partition per class; pads gather row 0 (S row is zero).
    nbatch = [-(-Cl[l] // NB) for l in (0, 1)]
    trim = os.environ.get("K_TRIM", "0") == "1"
    for l in (0, 1):
        g = gi[l]
        pad = g == PADG
        g[pad] = 0
        if not trim:
            continue
        # trailing pads of each batch marked idx=-1 (Q7 trims them)
        for b in range(nbatch[l]):
            e0 = min((b + 1) * NB, Cl[l]) * 128
            s0 = b * NB * 128
            for c in range(NCORES):
                run = 0
                while run < e0 - s0 and pad[c, e0 - 1 - run]:
                    run += 1
                if run:
                    g[c, e0 - run : e0] = -1

    def pack_idx(a):  # [C*128] -> [128, C*8], idx i at [i%16, i//16], repl x8
        half = a.reshape(-1, 16).T
        return np.tile(half, (8, 1)).astype(np.int16)

    def pack_drel(a, C):  # [C*128] -> [128, C]
        return np.ascontiguousarray(a.reshape(C, 128).T)

    # host-built S (one-hot) tensors: [128, Cl*128] bf16 where
    # S[p, j*128 + w] = 1 iff slot p of chunk j targets dst-rel w
    def build_s(drel_c, C):
        Sf = np.zeros((C * 128, 128), np.float32)
        rows = np.nonzero(drel_c < 128)[0]
        Sf[rows, drel_c[rows]] = 1.0
        return np.ascontiguousarray(
            Sf.reshape(C, 128, 128).transpose(1, 0, 2).reshape(128, C * 128)
        ).astype(BF16)

    dinv_new = np.zeros(Np, np.float32)
    dinv_new[newid] = dinv
    x_new = np.zeros((Np, INC), np.float32)
    x_new[newid] = x

    per_core = []
    for c in range(NCORES):
        d = {}
        d["idx0"] = pack_idx(gi[0][c])
        d["idx1"] = pack_idx(gi[1][c])
        if SGEN == "dma":
            d["s0"] = build_s(dr[0][c], Cl[0])
            d["s1"] = build_s(dr[1][c], Cl[1])
        else:
            d["drel0"] = pack_drel(dr[0][c], Cl[0])
            d["drel1"] = pack_drel(dr[1][c], Cl[1])
        d["dinv"] = np.ascontiguousarray(
            dinv_new[c * NPCp : (c + 1) * NPCp].reshape(NW, 128).T
        )
        d["xT"] = np.ascontiguousarray(x_new[c * NPCp : (c + 1) * NPCp].T)
        per_core.append(d)

    # per-window chunk lists and issue schedule
    window_chunks = []  # [w] -> list of (cls, chunk_j)
    for w in range(NW):
        lst = []
        for l in (0, 1):
            for j in range(base[w][l], base[w][l] + chmax[w][l]):
                lst.append((l, int(j)))
        window_chunks.append(lst)

    # chunk -> window map, batch trigger windows. Sort (trig, cls, fw):
    # same-trigger A-batches issue before B so the B gather's wait on
    # AG-B never starves the GPSIMD queue of A work.
    chunk_win = [np.repeat(np.arange(NW), chmax[:, l]) for l in (0, 1)]
    issue = []  # (trigger_w, cls, first_w, b)
    for l in (0, 1):
        ahead = AHEAD_A if l == 0 else AHEAD_B
        for b in range(nbatch[l]):
            fw = int(chunk_win[l][b * NB]) if b * NB < Cl[l] else NW - 1
            trig = max(0, fw - ahead)
            issue.append((trig, l, fw, b))
    issue.sort()

    meta = dict(
        N=N, Np=Np, NPC=NPC, NPCp=NPCp, NW=NW, NWA=NWA, NWB=NWB,
        NPCa=NPCa, NPCb=NPCb, chmax=chmax, base=base, Cl=Cl, INC=INC,
        nbatch=nbatch, window_chunks=window_chunks, issue=issue,
    )
    return meta, per_core, newid


def _build(nc, meta, has_b1, has_b2, has_b3):
    NW, NPCp = meta["NW"], meta["NPCp"]
    NWA, NPCa, NPCb = meta["NWA"], meta["NPCa"], meta["NPCb"]
    Cl, INC = meta["Cl"], meta["INC"]
    nbatch = meta["nbatch"]
    window_chunks = meta["window_chunks"]
    issue = meta["issue"]

    # ---- I/O -----------------------------------------------------------
    xT_d = nc.dram_tensor("xT", [INC, NPCp], F32, kind="ExternalInput")
    idx_d = [nc.dram_tensor(f"idx{l}", [128, Cl[l] * 8], I16,
                            kind="ExternalInput") for l in (0, 1)]
    if SGEN == "dma":
        s_d = [nc.dram_tensor(f"s{l}", [128, Cl[l] * 128], BF,
                              kind="ExternalInput") for l in (0, 1)]
    else:
        drel_d = [nc.dram_tensor(f"drel{l}", [128, Cl[l]], I16,
                                 kind="ExternalInput") for l in (0, 1)]
        iota_d = nc.dram_tensor("iota", [128, 128], I16, kind="ExternalInput")
    dinv_d = nc.dram_tensor("dinv", [128, NW], F32, kind="ExternalInput")
    w1_d = nc.dram_tensor("w1", [INC, CH], F32, kind="ExternalInput")
    w2_d = nc.dram_tensor("w2", [CH, CH], F32, kind="ExternalInput")
    w3_d = nc.dram_tensor("w3", [CH, OUTC], F32, kind="ExternalInput")
    id16_d = nc.dram_tensor("id16", [128, 128], BF, kind="ExternalInput")
    id32_d = nc.dram_tensor("id32", [128, 128], F32, kind="ExternalInput")
    b_d = {}
    if has_b1:
        b_d[1] = nc.dram_tensor("b1b", [128, CH], F32, kind="ExternalInput")
    if has_b2:
        b_d[2] = nc.dram_tensor("b2b", [128, CH], F32, kind="ExternalInput")
    if has_b3:
        b_d[3] = nc.dram_tensor("b3b", [128, OUTC], F32, kind="ExternalInput")
    out_d = nc.dram_tensor("out", [128, NW * OUTC], F32, kind="ExternalOutput")

    with tile.TileContext(nc) as tc:
        with (
            tc.tile_pool(name="const", bufs=1) as cpool,
            tc.tile_pool(name="hp", bufs=2) as hp_pool,
            tc.tile_pool(name="act", bufs=2) as act_pool,
            tc.tile_pool(name="xt", bufs=2) as xt_pool,
            tc.tile_pool(name="stage", bufs=4) as stg_pool,
            tc.tile_pool(name="msg0", bufs=BUFS_A) as msg_pool0,
            tc.tile_pool(name="msg1", bufs=BUFS_B) as msg_pool1,
            tc.tile_pool(name="sg0", bufs=BUFS_A) as s_pool0,
            tc.tile_pool(name="sg1", bufs=BUFS_B) as s_pool1,
            tc.tile_pool(name="aggps", bufs=4, space="PSUM") as agg_psum,
            tc.tile_pool(name="trps", bufs=2, space="PSUM") as tr_psum,
            tc.tile_pool(name="trxt", bufs=2, space="PSUM") as xt_psum,
            tc.tile_pool(name="dram", bufs=1, space="DRAM") as dram,
        ):
            # ---- residents ------------------------------------------------
            def load(shape, dtype, srct):
                t = cpool.tile(shape, dtype, tag=f"c_{srct.name}")
                nc.sync.dma_start(t[:], srct[:])
                return t

            # transform-critical residents first so layer 1 starts ASAP
            t_xT = load([INC, NPCp], F32, xT_d)
            t_w1 = load([INC, CH], F32, w1_d)
            t_dinv = load([128, NW], F32, dinv_d)
            t_w2 = load([CH, CH], F32, w2_d)
            t_w3 = load([CH, OUTC], F32, w3_d)
            t_id16 = load([128, 128], BF, id16_d)
            t_id32 = load([128, 128], F32, id32_d)
            t_b = {k: load(v.shape, F32, v) for k, v in b_d.items()}
            t_idx = [load([128, Cl[l] * 8], I16, idx_d[l]) for l in (0, 1)]
            if SGEN != "dma":
                t_drel = [load([128, Cl[l]], I16, drel_d[l]) for l in (0, 1)]
                t_iota = load([128, 128], I16, iota_d)

            def dinv_ap(t):
                return t_dinv[:][:, t : t + 1]

            # AG buffers: per layer, classes A and B
            ag = []
            for i in range(3):
                ai = dram.tile([NPCa, 128], BF, tag=f"agA_in{i}",
                               name=f"agA_in{i}")
                af = dram.tile([NPCa * NCORES, 128], BF, addr_space="Shared",
                               tag=f"agA_f{i}", name=f"agA_f{i}")
                bi = dram.tile([NPCb, 128], BF, tag=f"agB_in{i}",
                               name=f"agB_in{i}")
                bf_ = dram.tile([NPCb * NCORES, 128], BF, addr_space="Shared",
                                tag=f"agB_f{i}", name=f"agB_f{i}")
                ag.append(((ai, af), (bi, bf_)))

            def allgather(i, l):
                ag_in, ag_full = ag[i][l]
                nc.gpsimd.collective_compute(
                    "AllGather",
                    mybir.AluOpType.bypass,
                    replica_groups=[list(range(NCORES))],
                    ins=[ag_in[:].opt()],
                    outs=[ag_full[:].opt()],
                )

            def store_win(hp_tile, i, w):
                """Store window w's rows (bf16, cols 0:64) into ag_in."""
                l = 0 if w < NWA else 1
                wl = w if w < NWA else w - NWA
                ag_in = ag[i][l][0]
                with nc.allow_non_contiguous_dma("win store"):
                    nc.sync.dma_start(
                        ag_in[:][wl * 128 : (wl + 1) * 128, 0:CH],
                        hp_tile[:][:, w * CH : (w + 1) * CH],
                    )

            # ---------------------------------------------------------------
            def agg_layer(hp_tile, evict, i):
                """Aggregation for layer i reading ag[i], with explicit
                eager gather scheduling."""
                mtiles = [{}, {}]
                stiles = [{}, {}]
                ptr = [0]

                def issue_batch(l, b):
                    nb = min(NB, Cl[l] - b * NB)
                    q = (i * 37 + b * 2 + l) % 4
                    mpool = msg_pool0 if l == 0 else msg_pool1
                    spool = s_pool0 if l == 0 else s_pool1
                    mt = mpool.tile([128, NB, 128], BF, tag=f"msg{l}")
                    nc.gpsimd.dma_gather(
                        mt[:][:, :nb, :],
                        ag[i][l][1][:],
                        t_idx[l][:][:, b * NB * 8 : (b * NB + nb) * 8],
                        num_idxs=nb * 128,
                        num_idxs_reg=nb * 128,
                        elem_size=128,
                        elem_step=128,
                        single_packet=False,
                        queue_num=q,
                    )
                    st = spool.tile([128, NB * 128], BF, tag=f"sg{l}")
                    if SGEN == "dma":
                        nc.sync.dma_start(
                            st[:][:, : nb * 128],
                            s_d[l][:][:, b * NB * 128 : (b * NB + nb) * 128],
                        )
                    else:
                        nc.vector.tensor_tensor(
                            st[:].rearrange("p (c w) -> p c w", w=128)[:, :nb, :],
                            t_drel[l][:][:, b * NB : b * NB + nb][
                                :, :, None
                            ].broadcast_to([128, nb, 128]),
                            t_iota[:][:, None, :].broadcast_to([128, nb, 128]),
                            mybir.AluOpType.is_equal,
                        )
                    mtiles[l][b] = mt
                    stiles[l][b] = st

                # layer-start prefix: force the first A-batches out so the
                # B0 gather's wait on AG-B never starves the GPSIMD queue
                npre = min(PREFIX_A, nbatch[0], BUFS_A)
                for b in range(npre):
                    issue_batch(0, b)

                for w in range(NW):
                    while ptr[0] < len(issue) and issue[ptr[0]][0] <= w:
                        _, l, _, b = issue[ptr[0]]
                        if not (l == 0 and b < npre):
                            issue_batch(l, b)
                        ptr[0] += 1
                    chunks = window_chunks[w]
                    ps = agg_psum.tile([128, CH], F32, tag="aggps")
                    nc.tensor.matmul(
                        ps[:],
                        t_id16[:],
                        hp_tile[:][:, w * CH : (w + 1) * CH],
                        start=True,
                        stop=(not chunks),
                    )
                    nmm = len(chunks)
                    k = 0
                    for l, j in chunks:
                        b, loc = divmod(j, NB)
                        s_ap = stiles[l][b][:][:, loc * 128 : (loc + 1) * 128]
                        m_ap = mtiles[l][b][:][:, loc, 0:CH]
                        k += 1
                        nc.tensor.matmul(
                            ps[:], s_ap, m_ap, start=False, stop=(k == nmm)
                        )
                    evict(w, ps)

            # ---- layer 1: transform x @ W1, store + AG ----------------------
            hp1 = hp_pool.tile([128, NW * CH], BF, tag="hp")
            for w in range(NW):
                ps = tr_psum.tile([128, CH], F32, tag="trps")
                nc.tensor.matmul(
                    ps[:],
                    t_xT[:][:, w * 128 : (w + 1) * 128],
                    t_w1[:],
                    start=True,
                    stop=True,
                )
                nc.scalar.activation(
                    hp1[:][:, w * CH : (w + 1) * CH],
                    ps[:],
                    mybir.ActivationFunctionType.Copy,
                    bias=0.0,
                    scale=dinv_ap(w),
                )
                store_win(hp1, 0, w)
                if w == NWA - 1:
                    allgather(0, 0)
                if w == NW - 1:
                    allgather(0, 1)

            # ---- layer 1 aggregation + lrelu + transform to hp2 ------------
            act1 = act_pool.tile([128, NW * CH], F32, tag="act")
            hp2 = hp_pool.tile([128, NW * CH], BF, tag="hp")

            def evict_lrelu(act_tile, has_b, bkey):
                def _e(w, ps):
                    stg = stg_pool.tile([128, CH], F32, tag="stg")
                    if has_b:
                        nc.vector.scalar_tensor_tensor(
                            stg[:], ps[:], dinv_ap(w), t_b[bkey][:],
                            mybir.AluOpType.mult, mybir.AluOpType.add,
                        )
                    else:
                        nc.scalar.activation(
                            stg[:], ps[:],
                            mybir.ActivationFunctionType.Copy,
                            bias=0.0, scale=dinv_ap(w),
                        )
                    nc.vector.scalar_tensor_tensor(
                        act_tile[:][:, w * CH : (w + 1) * CH],
                        stg[:], 0.2, stg[:],
                        mybir.AluOpType.mult, mybir.AluOpType.max,
                    )
                return _e

            ev1 = evict_lrelu(act1, has_b1, 1)

            def evict1(w, ps):
                ev1(w, ps)
                psx = xt_psum.tile([CH, 128], F32, tag="trxt")
                nc.tensor.transpose(
                    psx[:], act1[:][:, w * CH : (w + 1) * CH], t_id32[:]
                )
                xt = xt_pool.tile([CH, 128], F32, tag="xt")
                nc.scalar.copy(xt[:], psx[:])
                ps2 = tr_psum.tile([128, CH], F32, tag="trps")
                nc.tensor.matmul(ps2[:], xt[:], t_w2[:], start=True, stop=True)
                nc.scalar.activation(
                    hp2[:][:, w * CH : (w + 1) * CH],
                    ps2[:],
                    mybir.ActivationFunctionType.Copy,
                    bias=0.0,
                    scale=dinv_ap(w),
                )
                store_win(hp2, 1, w)
                if w == NWA - 1:
                    allgather(1, 0)
                if w == NW - 1:
                    allgather(1, 1)

            agg_layer(hp1, evict1, 0)

            # ---- layer 2 aggregation; layer-3 prescale fused ----------------
            act2 = act_pool.tile([128, NW * CH], F32, tag="act")
            hp3 = hp_pool.tile([128, NW * CH], BF, tag="hp")
            ev2 = evict_lrelu(act2, has_b2, 2)

            def evict2(w, ps):
                ev2(w, ps)
                nc.scalar.activation(
                    hp3[:][:, w * CH : (w + 1) * CH],
                    act2[:][:, w * CH : (w + 1) * CH],
                    mybir.ActivationFunctionType.Copy,
                    bias=0.0,
                    scale=dinv_ap(w),
                )
                store_win(hp3, 2, w)
                if w == NWA - 1:
                    allgather(2, 0)
                if w == NW - 1:
                    allgather(2, 1)

            agg_layer(hp2, evict2, 1)

            # ---- layer 3 aggregation; output transform + tanh ---------------
            agg3 = act_pool.tile([128, NW * CH], F32, tag="act")
            outsb = cpool.tile([128, NW * OUTC], F32, tag="outsb")

            def evict3(w, ps):
                nc.scalar.activation(
                    agg3[:][:, w * CH : (w + 1) * CH],
                    ps[:],
                    mybir.ActivationFunctionType.Copy,
                    bias=0.0,
                    scale=dinv_ap(w),
                )
                psx = xt_psum.tile([CH, 128], F32, tag="trxt")
                nc.tensor.transpose(
                    psx[:], agg3[:][:, w * CH : (w + 1) * CH], t_id32[:]
                )
                xt = xt_pool.tile([CH, 128], F32, tag="xt")
                nc.scalar.copy(xt[:], psx[:])
                ps3 = tr_psum.tile([128, OUTC], F32, tag="trps")
                nc.tensor.matmul(ps3[:], xt[:], t_w3[:], start=True, stop=True)
                o_ap = outsb[:][:, w * OUTC : (w + 1) * OUTC]
                if has_b3:
                    stg = stg_pool.tile([128, OUTC], F32, tag="stgo")
                    nc.vector.tensor_add(stg[:], ps3[:], t_b[3][:])
                    nc.scalar.activation(
                        o_ap, stg[:], mybir.ActivationFunctionType.Tanh
                    )
                else:
                    nc.scalar.activation(
                        o_ap, ps3[:], mybir.ActivationFunctionType.Tanh
                    )

            agg_layer(hp3, evict3, 2)
            nc.sync.dma_start(out_d[:], outsb[:])

    nc.finalize()


def kernel(x, edge_index, W1, b1, W2, b2, W3, b3):
    global LAST_PERF
    x = np.asarray(x, np.float32)
    edge_index = np.asarray(edge_index)
    W1 = np.asarray(W1, np.float32)
    W2 = np.asarray(W2, np.float32)
    W3 = np.asarray(W3, np.float32)
    b1 = np.asarray(b1, np.float32)
    b2 = np.asarray(b2, np.float32)
    b3 = np.asarray(b3, np.float32)

    meta, per_core, newid = _prep(x, edge_index)
    has_b1 = bool(np.any(b1))
    has_b2 = bool(np.any(b2))
    has_b3 = bool(np.any(b3))

    if os.environ.get("BASS_TRACE"):
        _install_ntff_hook()

    nc = bacc.Bacc("TRN2", target_bir_lowering=False, debug=False,
                   num_devices=NCORES, num_swdge_queues=4)
    _build(nc, meta, has_b1, has_b2, has_b3)

    NW = meta["NW"]
    common = {
        "w1": W1, "w2": W2, "w3": W3,
        "id16": np.eye(128, dtype=BF16),
        "id32": np.eye(128, dtype=np.float32),
    }
    if SGEN != "dma":
        common["iota"] = np.broadcast_to(
            np.arange(128, dtype=np.int16), (128, 128)
        ).copy()
    if has_b1:
        common["b1b"] = np.broadcast_to(b1, (128, 64)).copy()
    if has_b2:
        common["b2b"] = np.broadcast_to(b2, (128, 64)).copy()
    if has_b3:
        common["b3b"] = np.broadcast_to(b3, (128, 16)).copy()

    in_maps = [{**per_core[c], **common} for c in range(NCORES)]
    res = run_bass_kernel_spmd(nc, in_maps, core_ids=list(range(NCORES)))
    LAST_PERF = res

    N = meta["N"]
    NPCp = meta["NPCp"]
    full = np.empty((meta["Np"], 16), np.float32)
    for c in range(NCORES):
        o = res.results[c]["out"]  # [128, NW*16]
        full[c * NPCp : (c + 1) * NPCp] = (
            o.reshape(128, NW, 16).transpose(1, 0, 2).reshape(NPCp, 16)
        )
    out = np.empty((N, 16), np.float32)
    out[:] = full[newid]
    return out


# revision 3
# speedup vs baseline: 1.3208x; 1.3208x over previous
"""3-layer GCN on 8 TRN2 NeuronCores — v2.

Changes vs v1 baseline (1.86ms):
  * bf16 features stored in 256B-padded rows ([Np, 128] bf16, cols 0:64
    meaningful) so each edge gather is ONE descriptor for ONE node row —
    drops the pair-row/parity-list machinery.
  * sources split into two halves (A = windows 0..23, B = 24..48) with
    separate AllGather buffers: keeps gather indices in int16 range AND
    lets AG-A fire mid-aggregation / AG-B at layer end, fully hidden
    behind gather descriptor generation (the GPSIMD wall).
  * S one-hot matrices precomputed on HOST and streamed from DRAM
    (eliminates 426us of DVE IS_EQ work).
  * eager, explicitly scheduled gather issue (A-prefix then interleave)
    with deep buffering to keep GPSIMD back-to-back.
  * per-window incremental stores of the next layer's features.
  * trailing pads of each gather batch marked idx=-1 (HW trims them).

Self-contained: only numpy/ml_dtypes + the concourse stack.
"""
import math
import os
import sys
import types

import numpy as np
import ml_dtypes

BF16 = ml_dtypes.bfloat16

for _p in ("/opt/trn_rl_repo",):
    if _p not in sys.path and os.path.isdir(_p):
        sys.path.insert(0, _p)

import concourse.bacc as bacc
import concourse.bass as bass
import concourse.mybir as mybir
from concourse import tile
from concourse.bass_utils import run_bass_kernel_spmd

F32 = mybir.dt.float32
BF = mybir.dt.bfloat16
I16 = mybir.dt.int16

NCORES = 8
W = 128
CH = 64
OUTC = 16
NB = int(os.environ.get("K_NB", "24"))      # chunks per gather batch
AHEAD_A = int(os.environ.get("K_AHEAD_A", "6"))   # windows of A-prefetch
AHEAD_B = int(os.environ.get("K_AHEAD_B", "2"))
BUFS_A = int(os.environ.get("K_BUFS_A", "6"))
BUFS_B = int(os.environ.get("K_BUFS_B", "3"))
PREFIX_A = int(os.environ.get("K_PREFIX_A", "6"))  # A-batches forced at layer start
SGEN = os.environ.get("K_SGEN", "ve")              # 've' = IS_EQ on DVE, 'dma' = host S

LAST_PERF = None


def _install_ntff_hook():
    if "antenv.axon_hooks" in sys.modules:
        return
    try:
        from trn_agent_boot.trn_boot import _ntff_profile_via_ctypes

        mod = types.ModuleType("antenv.axon_hooks")
        box = [None]
        mod.set_axon_ntff_profile_hook = lambda h: box.__setitem__(0, h)
        mod.get_axon_ntff_profile_hook = lambda: box[0]
        mod.set_axon_ntff_profile_hook(
            _ntff_profile_via_ctypes("/opt/axon/libaxon_pjrt.so")
        )
        sys.modules["antenv.axon_hooks"] = mod
    except Exception:
        pass


def _prep(x, edge_index):
    N = x.shape[0]
    E = edge_index.shape[1]
    INC = x.shape[1]
    NPC = N // NCORES                  # 6250
    NW = -(-NPC // W)                  # 49
    NPCp = NW * W                      # 6272
    Np = NPCp * NCORES                 # 50176
    NWA = NW // 2                      # 24 windows -> class A
    NWB = NW - NWA                     # 25 windows -> class B
    NPCa = NWA * W                     # 3072
    NPCb = NWB * W                     # 3200

    src = np.ascontiguousarray(edge_index[0]).astype(np.int64)
    dst = np.ascontiguousarray(edge_index[1]).astype(np.int64)

    deg = 1.0 + np.bincount(dst, minlength=N).astype(np.float64)
    dinv = (1.0 / np.sqrt(deg)).astype(np.float32)

    # degree-sorted round-robin deal (keeps per-window counts balanced
    # across cores): rank r -> core (r//W)%8, window r//(W*8)
    order = np.argsort(-deg, kind="stable")
    r = np.arange(N)
    new_of_rank = ((r // W) % NCORES) * NPCp + (r // (W * NCORES)) * W + (r % W)
    newid = np.empty(N, np.int64)
    newid[order] = new_of_rank

    s_new = newid[src]
    d_new = newid[dst]
    score, sloc = np.divmod(s_new, NPCp)
    cls = (sloc >= NPCa).astype(np.int64)          # 0=A, 1=B
    gidx = np.where(cls == 0, score * NPCa + sloc,
                    score * NPCb + (sloc - NPCa))
    core, dloc = np.divmod(d_new, NPCp)
    win = dloc // W
    rel = (dloc % W).astype(np.int16)

    # chunk schedule shared across cores (SPMD): per (win, cls)
    cnt = np.zeros((NCORES, NW, 2), np.int64)
    np.add.at(cnt, (core, win, cls), 1)
    chmax = -(-cnt.max(axis=0) // 128)             # [NW, 2]
    base = np.zeros((NW, 2), np.int64)
    Cl = [0, 0]
    for l in (0, 1):
        base[:, l] = np.cumsum(chmax[:, l]) - chmax[:, l]
        Cl[l] = int(chmax[:, l].sum())

    # slot assignment: vectorized cumcount within (core, cls, win)
    key = (core * 2 + cls) * NW + win
    o = np.argsort(key, kind="stable")
    ks = key[o]
    new_grp = np.empty(E, np.bool_)
    new_grp[0] = True
    new_grp[1:] = ks[1:] != ks[:-1]
    starts = np.nonzero(new_grp)[0]
    grp_of = np.cumsum(new_grp) - 1
    cumcount = np.arange(E) - starts[grp_of]
    slot_sorted = base[win[o], cls[o]] * 128 + cumcount

    PADG = np.int64(32767)
    gi = [np.full((NCORES, Cl[l] * 128), PADG, np.int64) for l in (0, 1)]
    dr = [np.full((NCORES, Cl[l] * 128), 128, np.int16) for l in (0, 1)]
    for l in (0, 1):
        m = cls[o] == l
        gi[l][core[o][m], slot_sorted[m]] = gidx[o][m]
        dr[l][core[o][m], slot_sorted[m]] = rel[o][m]

    # sort slots within each 128-chunk by gather index (HBM locality);
    # pads (gidx=32767) go last within the chunk
    for l in (0, 1):
        g3 = gi[l].reshape(NCORES, Cl[l], 128)
        d3 = dr[l].reshape(NCORES, Cl[l], 128)
        srt = np.argsort(g3, axis=2, kind="stable")
        gi[l] = np.take_along_axis(g3, srt, axis=2).reshape(NCORES, -1)
        dr[l] = np.take_along_axis(d3, srt, axis=2).reshape(NCORES, -1)

    # batch partition per class; pads gather row 0 (S row is zero).
    nbatch = [-(-Cl[l] // NB) for l in (0, 1)]
    trim = os.environ.get("K_TRIM", "0") == "1"
    for l in (0, 1):
        g = gi[l]
        pad = g == PADG
        g[pad] = 0
        if not trim:
            continue
        # trailing pads of each batch marked idx=-1 (Q7 trims them)
        for b in range(nbatch[l]):
            e0 = min((b + 1) * NB, Cl[l]) * 128
            s0 = b * NB * 128
            for c in range(NCORES):
                run = 0
                while run < e0 - s0 and pad[c, e0 - 1 - run]:
                    run += 1
                if run:
                    g[c, e0 - run : e0] = -1

    def pack_idx(a):  # [C*128] -> [128, C*8], idx i at [i%16, i//16], repl x8
        half = a.reshape(-1, 16).T
        return np.tile(half, (8, 1)).astype(np.int16)

    def pack_drel(a, C):  # [C*128] -> [128, C]
        return np.ascontiguousarray(a.reshape(C, 128).T)

    # host-built S (one-hot) tensors: [128, Cl*128] bf16 where
    # S[p, j*128 + w] = 1 iff slot p of chunk j targets dst-rel w
    def build_s(drel_c, C):
        Sf = np.zeros((C * 128, 128), np.float32)
        rows = np.nonzero(drel_c < 128)[0]
        Sf[rows, drel_c[rows]] = 1.0
        return np.ascontiguousarray(
            Sf.reshape(C, 128, 128).transpose(1, 0, 2).reshape(128, C * 128)
        ).astype(BF16)

    dinv_new = np.zeros(Np, np.float32)
    dinv_new[newid] = dinv
    x_new = np.zeros((Np, INC), np.float32)
    x_new[newid] = x

    per_core = []
    for c in range(NCORES):
        d = {}
        d["idx0"] = pack_idx(gi[0][c])
        d["idx1"] = pack_idx(gi[1][c])
        if SGEN == "dma":
            d["s0"] = build_s(dr[0][c], Cl[0])
            d["s1"] = build_s(dr[1][c], Cl[1])
        else:
            d["drel0"] = pack_drel(dr[0][c], Cl[0])
            d["drel1"] = pack_drel(dr[1][c], Cl[1])
        d["dinv"] = np.ascontiguousarray(
            dinv_new[c * NPCp : (c + 1) * NPCp].reshape(NW, 128).T
        )
        d["xT"] = np.ascontiguousarray(x_new[c * NPCp : (c + 1) * NPCp].T)
        per_core.append(d)

    # per-window chunk lists and issue schedule
    window_chunks = []  # [w] -> list of (cls, chunk_j)
    for w in range(NW):
        lst = []
        for l in (0, 1):
            for j in range(base[w][l], base[w][l] + chmax[w][l]):
                lst.append((l, int(j)))
        window_chunks.append(lst)

    # chunk -> window map, batch trigger windows. Sort (trig, cls, fw):
    # same-trigger A-batches issue before B so the B gather's wait on
    # AG-B never starves the GPSIMD queue of A work.
    chunk_win = [np.repeat(np.arange(NW), chmax[:, l]) for l in (0, 1)]
    issue = []  # (trigger_w, cls, first_w, b)
    for l in (0, 1):
        ahead = AHEAD_A if l == 0 else AHEAD_B
        for b in range(nbatch[l]):
            fw = int(chunk_win[l][b * NB]) if b * NB < Cl[l] else NW - 1
            trig = max(0, fw - ahead)
            issue.append((trig, l, fw, b))
    issue.sort()

    meta = dict(
        N=N, Np=Np, NPC=NPC, NPCp=NPCp, NW=NW, NWA=NWA, NWB=NWB,
        NPCa=NPCa, NPCb=NPCb, chmax=chmax, base=base, Cl=Cl, INC=INC,
        nbatch=nbatch, window_chunks=window_chunks, issue=issue,
    )
    return meta, per_core, newid


def _build(nc, meta, has_b1, has_b2, has_b3):
    NW, NPCp = meta["NW"], meta["NPCp"]
    NWA, NPCa, NPCb = meta["NWA"], meta["NPCa"], meta["NPCb"]
    Cl, INC = meta["Cl"], meta["INC"]
    nbatch = meta["nbatch"]
    window_chunks = meta["window_chunks"]
    issue = meta["issue"]

    # ---- I/O -----------------------------------------------------------
    xT_d = nc.dram_tensor("xT", [INC, NPCp], F32, kind="ExternalInput")
    idx_d = [nc.dram_tensor(f"idx{l}", [128, Cl[l] * 8], I16,
                            kind="ExternalInput") for l in (0, 1)]
    if SGEN == "dma":
        s_d = [nc.dram_tensor(f"s{l}", [128, Cl[l] * 128], BF,
                              kind="ExternalInput") for l in (0, 1)]
    else:
        drel_d = [nc.dram_tensor(f"drel{l}", [128, Cl[l]], I16,
                                 kind="ExternalInput") for l in (0, 1)]
        iota_d = nc.dram_tensor("iota", [128, 128], I16, kind="ExternalInput")
    dinv_d = nc.dram_tensor("dinv", [128, NW], F32, kind="ExternalInput")
    w1_d = nc.dram_tensor("w1", [INC, CH], F32, kind="ExternalInput")
    w2_d = nc.dram_tensor("w2", [CH, CH], F32, kind="ExternalInput")
    w3_d = nc.dram_tensor("w3", [CH, OUTC], F32, kind="ExternalInput")
    id16_d = nc.dram_tensor("id16", [128, 128], BF, kind="ExternalInput")
    id32_d = nc.dram_tensor("id32", [128, 128], F32, kind="ExternalInput")
    b_d = {}
    if has_b1:
        b_d[1] = nc.dram_tensor("b1b", [128, CH], F32, kind="ExternalInput")
    if has_b2:
        b_d[2] = nc.dram_tensor("b2b", [128, CH], F32, kind="ExternalInput")
    if has_b3:
        b_d[3] = nc.dram_tensor("b3b", [128, OUTC], F32, kind="ExternalInput")
    out_d = nc.dram_tensor("out", [128, NW * OUTC], F32, kind="ExternalOutput")

    with tile.TileContext(nc) as tc:
        with (
            tc.tile_pool(name="const", bufs=1) as cpool,
            tc.tile_pool(name="hp", bufs=2) as hp_pool,
            tc.tile_pool(name="act", bufs=2) as act_pool,
            tc.tile_pool(name="xt", bufs=2) as xt_pool,
            tc.tile_pool(name="stage", bufs=4) as stg_pool,
            tc.tile_pool(name="msg0", bufs=BUFS_A) as msg_pool0,
            tc.tile_pool(name="msg1", bufs=BUFS_B) as msg_pool1,
            tc.tile_pool(name="sg0", bufs=BUFS_A) as s_pool0,
            tc.tile_pool(name="sg1", bufs=BUFS_B) as s_pool1,
            tc.tile_pool(name="aggps", bufs=4, space="PSUM") as agg_psum,
            tc.tile_pool(name="trps", bufs=2, space="PSUM") as tr_psum,
            tc.tile_pool(name="trxt", bufs=2, space="PSUM") as xt_psum,
            tc.tile_pool(name="dram", bufs=1, space="DRAM") as dram,
        ):
            # ---- residents ------------------------------------------------
            def load(shape, dtype, srct):
                t = cpool.tile(shape, dtype, tag=f"c_{srct.name}")
                nc.sync.dma_start(t[:], srct[:])
                return t

            # transform-critical residents first so layer 1 starts ASAP
            t_xT = load([INC, NPCp], F32, xT_d)
            t_w1 = load([INC, CH], F32, w1_d)
            t_dinv = load([128, NW], F32, dinv_d)
            t_w2 = load([CH, CH], F32, w2_d)
            t_w3 = load([CH, OUTC], F32, w3_d)
            t_id16 = load([128, 128], BF, id16_d)
            t_id32 = load([128, 128], F32, id32_d)
            t_b = {k: load(v.shape, F32, v) for k, v in b_d.items()}
            t_idx = [load([128, Cl[l] * 8], I16, idx_d[l]) for l in (0, 1)]
            if SGEN != "dma":
                t_drel = [load([128, Cl[l]], I16, drel_d[l]) for l in (0, 1)]
                t_iota = load([128, 128], I16, iota_d)

            def dinv_ap(t):
                return t_dinv[:][:, t : t + 1]

            # AG buffers: per layer, classes A and B
            ag = []
            for i in range(3):
                ai = dram.tile([NPCa, 128], BF, tag=f"agA_in{i}",
                               name=f"agA_in{i}")
                af = dram.tile([NPCa * NCORES, 128], BF, addr_space="Shared",
                               tag=f"agA_f{i}", name=f"agA_f{i}")
                bi = dram.tile([NPCb, 128], BF, tag=f"agB_in{i}",
                               name=f"agB_in{i}")
                bf_ = dram.tile([NPCb * NCORES, 128], BF, addr_space="Shared",
                                tag=f"agB_f{i}", name=f"agB_f{i}")
                ag.append(((ai, af), (bi, bf_)))

            def allgather(i, l):
                ag_in, ag_full = ag[i][l]
                nc.gpsimd.collective_compute(
                    "AllGather",
                    mybir.AluOpType.bypass,
                    replica_groups=[list(range(NCORES))],
                    ins=[ag_in[:].opt()],
                    outs=[ag_full[:].opt()],
                )

            def store_win(hp_tile, i, w):
                """Store window w's rows (bf16, cols 0:64) into ag_in."""
                l = 0 if w < NWA else 1
                wl = w if w < NWA else w - NWA
                ag_in = ag[i][l][0]
                with nc.allow_non_contiguous_dma("win store"):
                    nc.sync.dma_start(
                        ag_in[:][wl * 128 : (wl + 1) * 128, 0:CH],
                        hp_tile[:][:, w * CH : (w + 1) * CH],
                    )

            # ---------------------------------------------------------------
            def agg_layer(hp_tile, evict, i):
                """Aggregation for layer i reading ag[i], with explicit
                eager gather scheduling."""
                mtiles = [{}, {}]
                stiles = [{}, {}]
                ptr = [0]

                def issue_batch(l, b):
                    nb = min(NB, Cl[l] - b * NB)
                    q = (i * 37 + b * 2 + l) % 4
                    mpool = msg_pool0 if l == 0 else msg_pool1
                    spool = s_pool0 if l == 0 else s_pool1
                    mt = mpool.tile([128, NB, 128], BF, tag=f"msg{l}")
                    nc.gpsimd.dma_gather(
                        mt[:][:, :nb, :],
                        ag[i][l][1][:],
                        t_idx[l][:][:, b * NB * 8 : (b * NB + nb) * 8],
                        num_idxs=nb * 128,
                        num_idxs_reg=nb * 128,
                        elem_size=128,
                        elem_step=128,
                        single_packet=False,
                        queue_num=q,
                    )
                    st = spool.tile([128, NB * 128], BF, tag=f"sg{l}")
                    if SGEN == "dma":
                        nc.sync.dma_start(
                            st[:][:, : nb * 128],
                            s_d[l][:][:, b * NB * 128 : (b * NB + nb) * 128],
                        )
                    else:
                        nc.vector.tensor_tensor(
                            st[:].rearrange("p (c w) -> p c w", w=128)[:, :nb, :],
                            t_drel[l][:][:, b * NB : b * NB + nb][
                                :, :, None
                            ].broadcast_to([128, nb, 128]),
                            t_iota[:][:, None, :].broadcast_to([128, nb, 128]),
                            mybir.AluOpType.is_equal,
                        )
                    mtiles[l][b] = mt
                    stiles[l][b] = st

                # layer-start prefix: force the first A-batches out so the
                # B0 gather's wait on AG-B never starves the GPSIMD queue
                npre = min(PREFIX_A, nbatch[0], BUFS_A)
                for b in range(npre):
                    issue_batch(0, b)

                for w in range(NW):
                    while ptr[0] < len(issue) and issue[ptr[0]][0] <= w:
                        _, l, _, b = issue[ptr[0]]
                        if not (l == 0 and b < npre):
                            issue_batch(l, b)
                        ptr[0] += 1
                    chunks = window_chunks[w]
                    ps = agg_psum.tile([128, CH], F32, tag="aggps")
                    nc.tensor.matmul(
                        ps[:],
                        t_id16[:],
                        hp_tile[:][:, w * CH : (w + 1) * CH],
                        start=True,
                        stop=(not chunks),
                    )
                    nmm = len(chunks)
                    k = 0
                    for l, j in chunks:
                        b, loc = divmod(j, NB)
                        s_ap = stiles[l][b][:][:, loc * 128 : (loc + 1) * 128]
                        m_ap = mtiles[l][b][:][:, loc, 0:CH]
                        k += 1
                        nc.tensor.matmul(
                            ps[:], s_ap, m_ap, start=False, stop=(k == nmm)
                        )
                    evict(w, ps)

            # ---- layer 1: transform x @ W1, store + AG ----------------------
            hp1 = hp_pool.tile([128, NW * CH], BF, tag="hp")
            for w in range(NW):
                ps = tr_psum.tile([128, CH], F32, tag="trps")
                nc.tensor.matmul(
                    ps[:],
                    t_xT[:][:, w * 128 : (w + 1) * 128],
                    t_w1[:],
                    start=True,
                    stop=True,
                )
                nc.scalar.activation(
                    hp1[:][:, w * CH : (w + 1) * CH],
                    ps[:],
                    mybir.ActivationFunctionType.Copy,
                    bias=0.0,
                    scale=dinv_ap(w),
                )
                store_win(hp1, 0, w)
                if w == NWA - 1:
                    allgather(0, 0)
                if w == NW - 1:
                    allgather(0, 1)

            # ---- layer 1 aggregation + lrelu + transform to hp2 ------------
            act1 = act_pool.tile([128, NW * CH], F32, tag="act")
            hp2 = hp_pool.tile([128, NW * CH], BF, tag="hp")

            def evict_lrelu(act_tile, has_b, bkey):
                def _e(w, ps):
                    stg = stg_pool.tile([128, CH], F32, tag="stg")
                    if has_b:
                        nc.vector.scalar_tensor_tensor(
                            stg[:], ps[:], dinv_ap(w), t_b[bkey][:],
                            mybir.AluOpType.mult, mybir.AluOpType.add,
                        )
                    else:
                        nc.scalar.activation(
                            stg[:], ps[:],
                            mybir.ActivationFunctionType.Copy,
                            bias=0.0, scale=dinv_ap(w),
                        )
                    nc.vector.scalar_tensor_tensor(
                        act_tile[:][:, w * CH : (w + 1) * CH],
                        stg[:], 0.2, stg[:],
                        mybir.AluOpType.mult, mybir.AluOpType.max,
                    )
                return _e

            ev1 = evict_lrelu(act1, has_b1, 1)

            def evict1(w, ps):
                ev1(w, ps)
                psx = xt_psum.tile([CH, 128], F32, tag="trxt")
                nc.tensor.transpose(
                    psx[:], act1[:][:, w * CH : (w + 1) * CH], t_id32[:]
                )
                xt = xt_pool.tile([CH, 128], F32, tag="xt")
                nc.scalar.copy(xt[:], psx[:])
                ps2 = tr_psum.tile([128, CH], F32, tag="trps")
                nc.tensor.matmul(ps2[:], xt[:], t_w2[:], start=True, stop=True)
                nc.scalar.activation(
                    hp2[:][:, w * CH : (w + 1) * CH],
                    ps2[:],
                    mybir.ActivationFunctionType.Copy,
                    bias=0.0,
                    scale=dinv_ap(w),
                )
                store_win(hp2, 1, w)
                if w == NWA - 1:
                    allgather(1, 0)
                if w == NW - 1:
                    allgather(1, 1)

            agg_layer(hp1, evict1, 0)

            # ---- layer 2 aggregation; layer-3 prescale fused ----------------
            act2 = act_pool.tile([128, NW * CH], F32, tag="act")
            hp3 = hp_pool.tile([128, NW * CH], BF, tag="hp")
            ev2 = evict_lrelu(act2, has_b2, 2)

            def evict2(w, ps):
                ev2(w, ps)
                nc.scalar.activation(
                    hp3[:][:, w * CH : (w + 1) * CH],
                    act2[:][:, w * CH : (w + 1) * CH],
                    mybir.ActivationFunctionType.Copy,
                    bias=0.0,
                    scale=dinv_ap(w),
                )
                store_win(hp3, 2, w)
                if w == NWA - 1:
                    allgather(2, 0)
                if w == NW - 1:
                    allgather(2, 1)

            agg_layer(hp2, evict2, 1)

            # ---- layer 3 aggregation; output transform + tanh ---------------
            agg3 = act_pool.tile([128, NW * CH], F32, tag="act")
            outsb = cpool.tile([128, NW * OUTC], F32, tag="outsb")

            def evict3(w, ps):
                nc.scalar.activation(
                    agg3[:][:, w * CH : (w + 1) * CH],
                    ps[:],
                    mybir.ActivationFunctionType.Copy,
                    bias=0.0,
                    scale=dinv_ap(w),
                )
                psx = xt_psum.tile([CH, 128], F32, tag="trxt")
                nc.tensor.transpose(
                    psx[:], agg3[:][:, w * CH : (w + 1) * CH], t_id32[:]
                )
                xt = xt_pool.tile([CH, 128], F32, tag="xt")
                nc.scalar.copy(xt[:], psx[:])
                ps3 = tr_psum.tile([128, OUTC], F32, tag="trps")
                nc.tensor.matmul(ps3[:], xt[:], t_w3[:], start=True, stop=True)
                o_ap = outsb[:][:, w * OUTC : (w + 1) * OUTC]
                if has_b3:
                    stg = stg_pool.tile([128, OUTC], F32, tag="stgo")
                    nc.vector.tensor_add(stg[:], ps3[:], t_b[3][:])
                    nc.scalar.activation(
                        o_ap, stg[:], mybir.ActivationFunctionType.Tanh
                    )
                else:
                    nc.scalar.activation(
                        o_ap, ps3[:], mybir.ActivationFunctionType.Tanh
                    )

            agg_layer(hp3, evict3, 2)
            nc.sync.dma_start(out_d[:], outsb[:])

    nc.finalize()


def kernel(x, edge_index, W1, b1, W2, b2, W3, b3):
    global LAST_PERF
    x = np.asarray(x, np.float32)
    edge_index = np.asarray(edge_index)
    W1 = np.asarray(W1, np.float32)
    W2 = np.asarray(W2, np.float32)
    W3 = np.asarray(W3, np.float32)
    b1 = np.asarray(b1, np.float32)
    b2 = np.asarray(b2, np.float32)
    b3 = np.asarray(b3, np.float32)

    meta, per_core, newid = _prep(x, edge_index)
    has_b1 = bool(np.any(b1))
    has_b2 = bool(np.any(b2))
    has_b3 = bool(np.any(b3))

    if os.environ.get("BASS_TRACE"):
        _install_ntff_hook()

    nc = bacc.Bacc("TRN2", target_bir_lowering=False, debug=False,
                   num_devices=NCORES, num_swdge_queues=4)
    _build(nc, meta, has_b1, has_b2, has_b3)

    NW = meta["NW"]
    common = {
        "w1": W1, "w2": W2, "w3": W3,
        "id16": np.eye(128, dtype=BF16),
        "id32": np.eye(128, dtype=np.float32),
    }
    if SGEN != "dma":
        common["iota"] = np.broadcast_to(
            np.arange(128, dtype=np.int16), (128, 128)
        ).copy()
    if has_b1:
        common["b1b"] = np.broadcast_to(b1, (128, 64)).copy()
    if has_b2:
        common["b2b"] = np.broadcast_to(b2, (128, 64)).copy()
    if has_b3:
        common["b3b"] = np.broadcast_to(b3, (128, 16)).copy()

    in_maps = [{**per_core[c], **common} for c in range(NCORES)]
    res = run_bass_kernel_spmd(nc, in_maps, core_ids=list(range(NCORES)))
    LAST_PERF = res

    N = meta["N"]
    NPCp = meta["NPCp"]
    full = np.empty((meta["Np"], 16), np.float32)
    for c in range(NCORES):
        o = res.results[c]["out"]  # [128, NW*16]
        full[c * NPCp : (c + 1) * NPCp] = (
            o.reshape(128, NW, 16).transpose(1, 0, 2).reshape(NPCp, 16)
        )
    out = np.empty((N, 16), np.float32)
    out[:] = full[newid]
    return out
